# revision 11
# baseline (speedup 1.0000x reference)
"""Trainium2 Bass kernel for MultiHeadSelfAttention + BN + residual + LeakyReLU.

Sharding: 8 cores = (batch b, token-half s); each core computes all 8 heads of
attention for its 1152 query tokens of its batch, the full output projection
for those tokens, and BN via a single all-core AllReduce of per-channel
sum/sumsq statistics.
"""

import sys
import types

if "/opt/trn_rl_repo" not in sys.path:
    sys.path.insert(0, "/opt/trn_rl_repo")

import numpy as np
import ml_dtypes

N_CORES = 8
B, C, HH, WW = 4, 256, 48, 48
L = HH * WW            # 2304 tokens per batch
NH, DK = 8, 64
D = NH * DK            # 512
LQ = L // 2            # 1152 query tokens per core
NKT = L // 128         # 18 key tiles
NQT = LQ // 128        # 9 query tiles
STREAM = NKT * LQ      # 20736 logit columns per head
EXP_OP = 1536          # columns per exp ACTIVATE (3 PSUM banks)
NSAMP = B * L          # 9216 samples per channel for BN
BN_EPS = 1e-5
NEG = 0.01

TRACE = False
DEBUG = False
VARIANT = "full"  # "noattn" | "evenpos" | "full"
STOP_AFTER = "full"  # "qkv" | "outproj" | "ar" | "full"

_cache = {}


def _accum_col(qt):
    # packed AV accumulator columns; avoid crossing the 512-col PSUM bank edge
    return qt * 65 if qt < 7 else 512 + (qt - 7) * 65


def _build():
    import concourse.bacc as bacc
    import concourse.mybir as mybir
    import concourse.tile as tile

    dt = mybir.dt
    f32, f32r, bf16 = dt.float32, dt.float32r, dt.bfloat16
    Alu = mybir.AluOpType

    nc = bacc.Bacc(trn_type="TRN2", num_devices=N_CORES, debug=False)

    # ---- DRAM I/O ----
    xf_d = nc.dram_tensor("xf", [C, L], f32r, kind="ExternalInput").ap()
    xq_d = nc.dram_tensor("xq", [C, LQ], f32r, kind="ExternalInput").ap()
    wqt_d = nc.dram_tensor("wqt", [C, D], f32r, kind="ExternalInput").ap()
    wkt_d = nc.dram_tensor("wkt", [C, D], f32r, kind="ExternalInput").ap()
    wvt_d = nc.dram_tensor("wvt", [C, NH * 65], f32r, kind="ExternalInput").ap()
    wot_d = nc.dram_tensor("wot", [D, C], bf16, kind="ExternalInput").ap()
    bnw_d = nc.dram_tensor("bnw2", [128, 2], f32, kind="ExternalInput").ap()
    bnb_d = nc.dram_tensor("bnb2", [128, 2], f32, kind="ExternalInput").ap()
    y_d = nc.dram_tensor("y", [C, LQ], f32, kind="ExternalOutput").ap()
    dbg = {}
    if DEBUG:
        dbg["k"] = nc.dram_tensor("dbg_k", [128, 4 * L], f32, kind="ExternalOutput").ap()
        dbg["q"] = nc.dram_tensor("dbg_q", [128, 4 * LQ], f32, kind="ExternalOutput").ap()
        dbg["v"] = nc.dram_tensor("dbg_v", [128, NKT * 520], bf16, kind="ExternalOutput").ap()
        dbg["at"] = nc.dram_tensor("dbg_at", [128, 4 * LQ], bf16, kind="ExternalOutput").ap()
        dbg["yp"] = nc.dram_tensor("dbg_yp", [C, LQ], f32, kind="ExternalOutput").ap()
        dbg["st"] = nc.dram_tensor("dbg_st", [128, 4], f32, kind="ExternalOutput").ap()

    with tile.TileContext(nc) as tc:
        with (
            tc.tile_pool(name="const", bufs=1) as cpool,
            tc.tile_pool(name="big", bufs=1) as big,
            tc.tile_pool(name="pair", bufs=2) as pairp,
            tc.tile_pool(name="scr", bufs=2) as scr,
            tc.tile_pool(name="psA", bufs=2, space="PSUM") as psA,
            tc.tile_pool(name="psB", bufs=1, space="PSUM") as psB,
            tc.tile_pool(name="dram", bufs=1, space="DRAM") as dram,
        ):
            # ---- SBUF constants / inputs ----
            xf_sb = cpool.tile([128, 2, L], f32r)
            xq_sb = cpool.tile([128, 2, LQ], f32r)
            wqt_sb = cpool.tile([128, 2, D], f32r)
            wkt_sb = cpool.tile([128, 2, D], f32r)
            wvt_sb = cpool.tile([128, 2, NH * 65], f32r)
            wot_sb = cpool.tile([128, 4, C], bf16)
            bnw_sb = cpool.tile([128, 2], f32)
            bnb_sb = cpool.tile([128, 2], f32)
            ones_sb = cpool.tile([128, 1], f32)

            for ct in range(2):
                nc.sync.dma_start(wkt_sb[:, ct, :], wkt_d[ct * 128:(ct + 1) * 128, :])
                nc.sync.dma_start(wqt_sb[:, ct, :], wqt_d[ct * 128:(ct + 1) * 128, :])
                nc.sync.dma_start(wvt_sb[:, ct, :], wvt_d[ct * 128:(ct + 1) * 128, :])
                nc.sync.dma_start(xq_sb[:, ct, :], xq_d[ct * 128:(ct + 1) * 128, :])
                # split xf column-wise for earlier availability of first chunks
                nc.sync.dma_start(xf_sb[:, ct, 0:L // 2],
                                  xf_d[ct * 128:(ct + 1) * 128, 0:L // 2])
                nc.sync.dma_start(xf_sb[:, ct, L // 2:L],
                                  xf_d[ct * 128:(ct + 1) * 128, L // 2:L])
            for dtl in range(4):
                nc.sync.dma_start(wot_sb[:, dtl, :], wot_d[dtl * 128:(dtl + 1) * 128, :])
            nc.sync.dma_start(bnw_sb[:], bnw_d[:])
            nc.sync.dma_start(bnb_sb[:], bnb_d[:])
            nc.vector.memset(ones_sb[:], 1.0)

            # ---- big SBUF tensors ----
            k_sb = big.tile([128, 4, L], f32r)        # K: [dpair, 128 rows=2 heads, kpos]
            q_sb = big.tile([128, 4, LQ], f32r)       # Q
            v_sb = big.tile([128, NKT, NH * 65], bf16)  # V token-major + ones cols
            expt = big.tile([128, STREAM], bf16)      # per-head exp stream (reused)
            attnT = big.tile([128, 4, LQ], bf16)      # transposed attention output
            y_sb = big.tile([128, 2, LQ], f32)
            stats = big.tile([128, 4], f32)
            gstats = big.tile([128, 4], f32)

            # ================= QKV background units =================
            def emit_k_half(dtl, half):
                ps = psA.tile([128, LQ], f32, name="qkvps", tag="stag")
                for ct in range(2):
                    for (q0, qn) in ((0, 512), (512, 512), (1024, 128)):
                        nc.tensor.matmul(
                            ps[:, q0:q0 + qn],
                            wkt_sb[:, ct, dtl * 128:(dtl + 1) * 128],
                            xf_sb[:, ct, half * LQ + q0: half * LQ + q0 + qn],
                            start=(ct == 0), stop=(ct == 1))
                nc.vector.tensor_copy(k_sb[:, dtl, half * LQ:(half + 1) * LQ], ps[:])

            def emit_q_tile(dtl):
                ps = psA.tile([128, LQ], f32, name="qkvps", tag="stag")
                for ct in range(2):
                    for (q0, qn) in ((0, 512), (512, 512), (1024, 128)):
                        nc.tensor.matmul(
                            ps[:, q0:q0 + qn],
                            wqt_sb[:, ct, dtl * 128:(dtl + 1) * 128],
                            xq_sb[:, ct, q0:q0 + qn],
                            start=(ct == 0), stop=(ct == 1))
                nc.vector.tensor_copy(q_sb[:, dtl, :], ps[:])

            def emit_v_ltile(lt):
                ps = psA.tile([128, LQ], f32, name="qkvps", tag="stag")
                for ct in range(2):
                    for (q0, qn) in ((0, 512), (512, 8)):
                        nc.tensor.matmul(
                            ps[:, q0:q0 + qn],
                            xf_sb[:, ct, lt * 128:(lt + 1) * 128],
                            wvt_sb[:, ct, q0:q0 + qn],
                            start=(ct == 0), stop=(ct == 1))
                nc.vector.tensor_copy(v_sb[:, lt, :], ps[:, 0:520])
                # ones columns for the softmax denominator
                nc.vector.memset(v_sb[:, lt, 64::65], 1.0)

            background = []
            for lt in range(2, NKT):
                background.append(lambda lt=lt: emit_v_ltile(lt))
            for dtl in range(1, 4):
                background.append(lambda d=dtl: emit_k_half(d, 0))
                background.append(lambda d=dtl: emit_k_half(d, 1))
                background.append(lambda d=dtl: emit_q_tile(d))

            # prologue: what head 0 needs immediately
            emit_k_half(0, 0)
            emit_k_half(0, 1)
            emit_q_tile(0)
            emit_v_ltile(0)
            emit_v_ltile(1)

            # ================= attention =================
            n_ops = (STREAM + EXP_OP - 1) // EXP_OP  # 14 (last op = 768 cols)

            heads = [] if VARIANT == "noattn" else list(range(NH))
            if VARIANT == "noattn":
                nc.vector.memset(attnT[:].rearrange("p a b -> p (a b)"), 0.0)
            for h in heads:
                pr = h // 2          # which 128-row pair tile
                r0 = 0 if VARIANT == "evenpos" else (h % 2) * 64
                tpos = (r0, 0)
                accum = psB.tile([128, 642], f32, name="avacc", tag="avacc")
                # matmul start=True clears the whole bank's has_written bits,
                # which would wipe sibling accumulation groups packed in the
                # same bank -- zero the values and accumulate with start=False.
                nc.vector.memset(accum[:], 0.0)
                av_done = 0
                bg_budget = 2.0 if h < 4 else 0.5
                bg_carry = 0.0

                for s in range(n_ops):
                    base = s * EXP_OP
                    end = min(base + EXP_OP, STREAM)
                    stag = psA.tile([128, EXP_OP], f32, name="stag", tag="stag")
                    # segment boundaries: PSUM 512-bank grid + ktile (1152) grid
                    pts = sorted({base, end}
                                 | set(range(base + 512, end, 512))
                                 | {k * LQ for k in range(1, NKT)
                                    if base < k * LQ < end})
                    segs = list(zip(pts, pts[1:]))
                    segs.sort(key=lambda ab: (ab[0] // LQ, ab[0]))
                    for (a, b) in segs:
                        t, q0 = a // LQ, a % LQ
                        nc.tensor.matmul(
                            stag[:, a - base:b - base],
                            k_sb[r0:r0 + 64, pr, t * 128:(t + 1) * 128],
                            q_sb[r0:r0 + 64, pr, q0:q0 + (b - a)],
                            start=True, stop=True, tile_position=tpos)
                    nc.scalar.activation(
                        expt[:, base:end], stag[:, 0:end - base],
                        mybir.ActivationFunctionType.Exp, scale=1.0 / np.sqrt(DK))
                    # AV for every ktile fully covered by the exp stream so far
                    while (av_done + 1) * LQ <= end:
                        t = av_done
                        for qt in range(NQT):
                            c0 = _accum_col(qt)
                            nc.tensor.matmul(
                                accum[:, c0:c0 + 65],
                                expt[:, t * LQ + qt * 128: t * LQ + (qt + 1) * 128],
                                v_sb[:, t, h * 65:(h + 1) * 65],
                                start=False, stop=(t == NKT - 1),
                                skip_group_check=True)
                        av_done += 1
                    bg_carry += bg_budget
                    while background and bg_carry >= 1.0:
                        background.pop(0)()
                        bg_carry -= 1.0

                # normalize: split packed accumulator at the bank edge
                attn_pair = pairp.tile([128, NQT, 128], bf16, name="apair", tag="apair") \
                    if h % 2 == 0 else attn_pair
                recA = scr.tile([128, 7], f32, name="recA", tag="recA")
                recB = scr.tile([128, 2], f32, name="recB", tag="recB")
                nc.vector.reciprocal(recA[:], accum[:, 64:64 + 7 * 65:65])
                nc.vector.reciprocal(recB[:], accum[:, 512 + 64:512 + 2 * 65:65])
                accA = accum[:, 0:7 * 65].rearrange("p (q d) -> p q d", d=65)[:, :, 0:64]
                accB = accum[:, 512:512 + 2 * 65].rearrange("p (q d) -> p q d", d=65)[:, :, 0:64]
                nc.vector.tensor_tensor(
                    attn_pair[:, 0:7, r0:r0 + 64], accA,
                    recA[:].unsqueeze(2).broadcast_to([128, 7, 64]), Alu.mult)
                nc.vector.tensor_tensor(
                    attn_pair[:, 7:9, r0:r0 + 64], accB,
                    recB[:].unsqueeze(2).broadcast_to([128, 2, 64]), Alu.mult)
                if h % 2 == 1:
                    for qt in range(NQT):
                        nc.sync.dma_start_transpose(
                            attnT[:, pr, qt * 128:(qt + 1) * 128],
                            attn_pair[:, qt, :])

            while background:
                background.pop(0)()

            # ================= output projection + stats =================
            if STOP_AFTER == "qkv":
                nc.vector.memset(stats[:], 0.5)
                nc.vector.memset(y_sb[:].rearrange("p a b -> p (a b)"), 0.5)
            for ct in ([] if STOP_AFTER == "qkv" else range(2)):
                ps = psA.tile([128, LQ], f32, name="yps", tag="stag")
                for dtl in range(4):
                    for (q0, qn) in ((0, 512), (512, 512), (1024, 128)):
                        nc.tensor.matmul(
                            ps[:, q0:q0 + qn],
                            wot_sb[:, dtl, ct * 128:(ct + 1) * 128],
                            attnT[:, dtl, q0:q0 + qn],
                            start=(dtl == 0), stop=(dtl == 3))
                sq = scr.tile([128, LQ], f32, name="sq", tag="sq")
                nc.vector.tensor_copy(y_sb[:, ct, :], ps[:])
                nc.vector.tensor_reduce(
                    stats[:, 2 * ct:2 * ct + 1], y_sb[:, ct, :],
                    mybir.AxisListType.X, Alu.add)
                nc.vector.tensor_tensor(sq[:], y_sb[:, ct, :], y_sb[:, ct, :], Alu.mult)
                nc.vector.tensor_reduce(
                    stats[:, 2 * ct + 1:2 * ct + 2], sq[:],
                    mybir.AxisListType.X, Alu.add)

            # ================= AllReduce of stats =================
            skip_ar = STOP_AFTER in ("outproj",)
            cin = dram.tile([128, 4], f32)
            cout = dram.tile([128, 4], f32, addr_space="Shared")
            if not skip_ar:
                nc.sync.dma_start(cin[:], stats[:])
                nc.gpsimd.collective_compute(
                    "AllReduce", Alu.add,
                    replica_groups=[list(range(N_CORES))],
                    ins=[cin.opt()], outs=[cout.opt()])
                nc.sync.dma_start(gstats[:], cout[:])
            else:
                nc.vector.tensor_copy(gstats[:], stats[:])

            # ================= BN coefficients =================
            mean = scr.tile([128, 2], f32, name="mean")
            m2 = scr.tile([128, 2], f32, name="m2")
            var = scr.tile([128, 2], f32, name="var")
            sd = scr.tile([128, 2], f32, name="sd")
            rstd = scr.tile([128, 2], f32, name="rstd")
            Ac = scr.tile([128, 2], f32, name="Ac")
            Bc = scr.tile([128, 2], f32, name="Bc")
            gs = gstats[:].rearrange("p (c two) -> p c two", two=2)
            nc.vector.tensor_scalar(mean[:], gs[:, :, 0], 1.0 / NSAMP, None, Alu.mult)
            nc.vector.tensor_scalar(m2[:], gs[:, :, 1], 1.0 / NSAMP, None, Alu.mult)
            nc.vector.tensor_tensor(var[:], mean[:], mean[:], Alu.mult)
            nc.vector.tensor_tensor(var[:], m2[:], var[:], Alu.subtract)
            nc.vector.tensor_scalar(var[:], var[:], BN_EPS, None, Alu.add)
            nc.scalar.activation(sd[:], var[:], mybir.ActivationFunctionType.Sqrt)
            nc.vector.reciprocal(rstd[:], sd[:])
            nc.vector.tensor_tensor(Ac[:], bnw_sb[:], rstd[:], Alu.mult)
            nc.vector.tensor_tensor(Bc[:], mean[:], Ac[:], Alu.mult)
            nc.vector.tensor_tensor(Bc[:], bnb_sb[:], Bc[:], Alu.subtract)

            # ================= apply + residual + leaky relu =================
            xq_f = xq_sb[:].bitcast(f32)
            for ct in range(2):
                z = scr.tile([128, LQ], f32, name="z")
                t2 = scr.tile([128, LQ], f32, name="t2")
                nc.vector.tensor_scalar(
                    z[:], y_sb[:, ct, :], Ac[:, ct:ct + 1], Bc[:, ct:ct + 1],
                    Alu.mult, Alu.add)
                nc.vector.tensor_tensor(z[:], z[:], xq_f[:, ct, :], Alu.add)
                nc.vector.tensor_scalar(t2[:], z[:], NEG, None, Alu.mult)
                nc.vector.tensor_tensor(z[:], z[:], t2[:], Alu.max)
                nc.sync.dma_start(y_d[ct * 128:(ct + 1) * 128, :], z[:])

            if DEBUG:
                nc.sync.dma_start(dbg["k"][:], k_sb[:].bitcast(f32).rearrange("p a b -> p (a b)"))
                nc.sync.dma_start(dbg["q"][:], q_sb[:].bitcast(f32).rearrange("p a b -> p (a b)"))
                nc.sync.dma_start(dbg["v"][:], v_sb[:].rearrange("p a b -> p (a b)"))
                nc.sync.dma_start(dbg["at"][:], attnT[:].rearrange("p a b -> p (a b)"))
                for ct in range(2):
                    nc.sync.dma_start(dbg["yp"][ct * 128:(ct + 1) * 128, :], y_sb[:, ct, :])
                nc.sync.dma_start(dbg["st"][:], gstats[:])

    nc.compile()
    return nc


def _prep_inputs(x, Wq, Wk, Wv, Wo, bn_w, bn_b, gamma):
    x = np.asarray(x, np.float32)
    Wq = np.asarray(Wq, np.float32)
    Wk = np.asarray(Wk, np.float32)
    Wv = np.asarray(Wv, np.float32)
    Wo = np.asarray(Wo, np.float32)
    bn_w = np.asarray(bn_w, np.float32)
    bn_b = np.asarray(bn_b, np.float32)
    gamma = np.asarray(gamma, np.float32)

    xf = x.reshape(B, C, L)
    wqt = np.ascontiguousarray(Wq.T)
    wkt = np.ascontiguousarray(Wk.T)
    wvt = np.zeros((C, NH * 65), np.float32)
    wvtT = Wv.T  # [C, D]
    for h in range(NH):
        wvt[:, h * 65:h * 65 + 64] = wvtT[:, h * 64:(h + 1) * 64]
    wot = np.ascontiguousarray(Wo.T).astype(ml_dtypes.bfloat16)
    g = float(gamma[0])
    bnw2 = np.ascontiguousarray((g * bn_w).reshape(2, 128).T)
    bnb2 = np.ascontiguousarray((g * bn_b).reshape(2, 128).T)

    in_maps = []
    for c in range(N_CORES):
        b, s = c // 2, c % 2
        in_maps.append({
            "xf": np.ascontiguousarray(xf[b]),
            "xq": np.ascontiguousarray(xf[b][:, s * LQ:(s + 1) * LQ]),
            "wqt": wqt, "wkt": wkt, "wvt": wvt, "wot": wot,
            "bnw2": bnw2, "bnb2": bnb2,
        })
    return in_maps


def kernel(x, Wq, Wk, Wv, Wo, bn_w, bn_b, gamma):
    # NTFF profile hook (needed only when TRACE=True, harmless otherwise)
    if "antenv.axon_hooks" not in sys.modules:
        try:
            import trn_agent_boot.trn_boot as _tb
            _h = _tb._ntff_profile_via_ctypes("/opt/axon/libaxon_pjrt.so")
            _m = types.ModuleType("antenv.axon_hooks")
            _m.get_axon_ntff_profile_hook = lambda: _h
            _m.set_axon_ntff_profile_hook = lambda hh: None
            sys.modules["antenv.axon_hooks"] = _m
        except Exception:
            pass

    from concourse import bass_utils

    if "nc" not in _cache:
        _cache["nc"] = _build()
    nc = _cache["nc"]

    in_maps = _prep_inputs(x, Wq, Wk, Wv, Wo, bn_w, bn_b, gamma)
    res = bass_utils.run_bass_kernel_spmd(
        nc, in_maps, core_ids=list(range(N_CORES)), trace=TRACE)
    _cache["last_result"] = res

    out = np.empty((B, C, L), np.float32)
    for c in range(N_CORES):
        b, s = c // 2, c % 2
        out[b][:, s * LQ:(s + 1) * LQ] = res.results[c]["y"]
    return out.reshape(B, C, HH, WW)
